# revision 24
# baseline (speedup 1.0000x reference)
"""Chebyshev-distance conv2d (p=inf "Conv2d") Trainium2 kernel.

Problem: y[b,o,ho,wo] = max_k |patch[b,k,ho,wo] - wf[o,k]|,
  B=8, C=32, O=64, H=W=48, 3x3 kernel, stride 1, pad 1, K = C*9 = 288.

Strategy (8 NeuronCores, data-parallel over batch, 1 image per core):
  - Partition dim (128) = 64 output channels x 2 spatial halves
    (rows 0..23 on partitions 0..63, rows 24..47 on partitions 64..127).
  - TensorE broadcasts each padded input-channel slab (26 rows x 50 cols,
    one slab per half) to all 128 partitions with a ones-matmul
    (contraction dim 2) into PSUM, double buffered.
  - ScalarE computes |w[o,k] - x| via activation(Abs, scale=-1,
    bias=w[o,k] per partition) reading tap-shifted views of the PSUM slab.
  - VectorE max-accumulates into the fp32 accumulator.
"""

import sys

if "/opt/trn_rl_repo" not in sys.path:
    sys.path.insert(0, "/opt/trn_rl_repo")

import numpy as np

import concourse.bass as bass
import concourse.bacc as bacc
import concourse.mybir as mybir
from concourse.tile import TileContext
from concourse.bass_utils import run_bass_kernel_spmd

B, C, O, H, W = 8, 32, 64, 48, 48
KS, PAD = 3, 1
HO, WO = 48, 48
K = C * KS * KS          # 288
NHALF = HO // 2          # 24 output rows per half
SLAB_R, SLAB_C = NHALF + 2, W + 2   # 26 x 50 padded slab per half
SLAB = SLAB_R * SLAB_C   # 1300
CGROUPS, CPG = 4, 8      # channel groups of 8 -> staging partitions = 2*4=8
# Channels whose |x-w| + max runs entirely on VectorE (load balance vs ScalarE)
DVE_CHANNELS = frozenset({3, 10, 16, 22, 28})

F32 = mybir.dt.float32
BF16 = mybir.dt.bfloat16


def build_nc():
    nc = bacc.Bacc(trn_type="TRN2")

    x_slab = nc.declare_dram_parameter(
        "x_slab", [2, C, SLAB_R, SLAB_C], F32, isOutput=False
    )
    wbias = nc.declare_dram_parameter("wbias", [128, K], F32, isOutput=False)
    ones2 = nc.declare_dram_parameter("ones2", [2, 128], F32, isOutput=False)
    out = nc.declare_dram_parameter("out", [128, NHALF, WO], F32, isOutput=True)

    with TileContext(nc) as tc:
        with (
            tc.tile_pool(name="const", bufs=1) as cpool,
            tc.tile_pool(name="work", bufs=3) as wpool,
            tc.tile_pool(name="psum", bufs=1, space="PSUM") as ppool,
        ):
            wb = cpool.tile([128, K], F32)
            ones = cpool.tile([2, 128], F32)
            # Three stacked max-accumulators (one TT covers a 3-tap batch);
            # acc_lo tracks min(x-w) for the VectorE-only channels.
            acc3 = cpool.tile([128, 3, NHALF, WO], BF16)
            acc_lo = cpool.tile([128, NHALF, WO], BF16)
            accf = cpool.tile([128, NHALF, WO], BF16)
            acc32 = cpool.tile([128, NHALF, WO], F32)
            stage = cpool.tile([2, C, SLAB_R, SLAB_C], F32)
            # Two persistent PSUM slabs, manually alternated per channel —
            # rotating pool slots would put >1 sem wait on the matmuls
            # (walrus allows only one on LDWEIGHTS).
            slab_a = ppool.tile([128, SLAB_R, SLAB_C], F32, tag="slab_a")
            slab_b = ppool.tile([128, SLAB_R, SLAB_C], F32, tag="slab_b")
            slabs = [slab_a, slab_b]

            # Single-queue SWDGE loads so consumers wait on one DMA sem.
            nc.gpsimd.dma_start(stage[:], x_slab[:])
            nc.gpsimd.dma_start(wb[:], wbias[:])
            nc.gpsimd.dma_start(ones[:], ones2[:])
            nc.vector.memset(acc3[:], 0.0)
            nc.vector.memset(acc_lo[:], 0.0)

            # Dummy 1-column matmul: absorbs the `ones` DMA wait on PE so
            # real matmuls carry at most one sem wait (LDWEIGHTS limit).
            slab0_f = slabs[0].rearrange("p r c -> p (r c)")
            nc.tensor.matmul(
                slab0_f[:, 0:1], ones[:], ones[:, 0:1], start=True, stop=True
            )

            for c in range(C):
                slab = slabs[c % 2]
                slab_f = slab.rearrange("p r c -> p (r c)")
                rhs = stage[:, c].rearrange("p r c -> p (r c)")
                # Broadcast both halves of channel c to the 128 partitions.
                for off in range(0, SLAB, 512):
                    sz = min(512, SLAB - off)
                    nc.tensor.matmul(
                        slab_f[:, off : off + sz],
                        ones[:],
                        rhs[:, off : off + sz],
                        start=True,
                        stop=True,
                    )
                if c in DVE_CHANNELS:
                    # VectorE-only: acc_hi = max(x-w, acc_hi),
                    # acc_lo = min(x-w, acc_lo); |x-w| folded at the end.
                    for tap in range(KS * KS):
                        kh, kw = tap // KS, tap % KS
                        k = c * (KS * KS) + tap
                        view = slab[:, kh : kh + NHALF, kw : kw + WO]
                        nc.vector.scalar_tensor_tensor(
                            acc3[:, 0],
                            view,
                            wb[:, k : k + 1],
                            acc3[:, 0],
                            op0=mybir.AluOpType.subtract,
                            op1=mybir.AluOpType.max,
                        )
                        nc.vector.scalar_tensor_tensor(
                            acc_lo[:],
                            view,
                            wb[:, k : k + 1],
                            acc_lo[:],
                            op0=mybir.AluOpType.subtract,
                            op1=mybir.AluOpType.min,
                        )
                else:
                    for t0 in range(0, KS * KS, 3):
                        tmp3 = wpool.tile([128, 3, NHALF, WO], BF16, tag="tmp")
                        for j in range(3):
                            tap = t0 + j
                            kh, kw = tap // KS, tap % KS
                            k = c * (KS * KS) + tap
                            # tmp3[:,j] = |w[o,k] - x_tap|
                            nc.scalar.activation(
                                tmp3[:, j],
                                slab[:, kh : kh + NHALF, kw : kw + WO],
                                mybir.ActivationFunctionType.Abs,
                                bias=wb[:, k : k + 1],
                                scale=-1.0,
                            )
                        nc.vector.tensor_tensor(
                            acc3[:], acc3[:], tmp3[:], op=mybir.AluOpType.max
                        )

            # y = max(acc3[0..2], -acc_lo)
            nc.vector.tensor_tensor(
                accf[:], acc3[:, 0], acc3[:, 1], op=mybir.AluOpType.max
            )
            nc.vector.tensor_tensor(
                accf[:], accf[:], acc3[:, 2], op=mybir.AluOpType.max
            )
            nc.vector.scalar_tensor_tensor(
                accf[:],
                acc_lo[:],
                -1.0,
                accf[:],
                op0=mybir.AluOpType.mult,
                op1=mybir.AluOpType.max,
            )
            nc.scalar.copy(acc32[:], accf[:])
            nc.sync.dma_start(out[:], acc32[:])

    nc.compile()
    return nc


_NC_CACHE = {}


def _get_nc():
    if "nc" not in _NC_CACHE:
        _NC_CACHE["nc"] = build_nc()
    return _NC_CACHE["nc"]


def make_in_maps(inputs: np.ndarray, weights: np.ndarray):
    x = np.asarray(inputs, dtype=np.float32)
    w = np.asarray(weights, dtype=np.float32)
    assert x.shape == (B, C, H, W) and w.shape == (O, C, KS, KS)

    xp = np.zeros((B, C, H + 2 * PAD, W + 2 * PAD), np.float32)
    xp[:, :, PAD : PAD + H, PAD : PAD + W] = x
    half_a = xp[:, :, 0:SLAB_R, :]                    # (B, C, 26, 50)
    half_b = xp[:, :, NHALF : NHALF + SLAB_R, :]      # (B, C, 26, 50)
    halves = np.stack([half_a, half_b], axis=2)       # (B, C, 2, 26, 50)
    stage = halves.transpose(0, 2, 1, 3, 4)           # (B, 2, C, 26, 50)

    wf = w.reshape(O, K)
    wb = np.ascontiguousarray(np.tile(wf, (2, 1)))    # (128, K)
    ones2 = np.zeros((2, 128), np.float32)
    ones2[0, :64] = 1.0
    ones2[1, 64:] = 1.0

    return [
        {
            "x_slab": np.ascontiguousarray(stage[b]),
            "wbias": wb,
            "ones2": ones2,
        }
        for b in range(B)
    ]


def assemble_output(results):
    y = np.empty((B, O, HO, WO), np.float32)
    for b in range(B):
        o = results[b]["out"]
        y[b, :, :NHALF, :] = o[0:64]
        y[b, :, NHALF:, :] = o[64:128]
    return y


def launch(inputs: np.ndarray, weights: np.ndarray, trace: bool = False):
    """Run on 8 NeuronCores; returns (y, BassKernelResults)."""
    in_maps = make_in_maps(inputs, weights)
    res = run_bass_kernel_spmd(
        _get_nc(), in_maps, list(range(B)), trace=trace
    )
    return assemble_output(res.results), res


def kernel(inputs: np.ndarray, weights: np.ndarray) -> np.ndarray:
    y, _ = launch(inputs, weights, trace=False)
    return y


# revision 32
# speedup vs baseline: 1.2156x; 1.2156x over previous
"""Chebyshev-distance conv2d (p=inf "Conv2d") Trainium2 kernel.

Problem: y[b,o,ho,wo] = max_k |patch[b,k,ho,wo] - wf[o,k]|,
  B=8, C=32, O=64, H=W=48, 3x3 kernel, stride 1, pad 1, K = C*9 = 288.

Strategy (8 NeuronCores, data-parallel over batch, 1 image per core):
  - Partition dim (128) = 64 output channels x 2 spatial halves
    (rows 0..23 on partitions 0..63, rows 24..47 on partitions 64..127).
  - TensorE broadcasts each padded input-channel slab (26 rows x 50 cols,
    one slab per half) to all 128 partitions with a ones-matmul
    (contraction dim 2) into PSUM, double buffered.
  - ScalarE computes |w[o,k] - x| via activation(Abs, scale=-1,
    bias=w[o,k] per partition) reading tap-shifted views of the PSUM slab.
  - VectorE max-accumulates into the fp32 accumulator.
"""

import sys

if "/opt/trn_rl_repo" not in sys.path:
    sys.path.insert(0, "/opt/trn_rl_repo")

import numpy as np

import concourse.bass as bass
import concourse.bacc as bacc
import concourse.mybir as mybir
from concourse.tile import TileContext
from concourse.bass_utils import run_bass_kernel_spmd

B, C, O, H, W = 8, 32, 64, 48, 48
KS, PAD = 3, 1
HO, WO = 48, 48
K = C * KS * KS          # 288
NHALF = HO // 2          # 24 output rows per half
SLAB_R, SLAB_C = NHALF + 2, W + 2   # 26 x 50 padded slab per half
SLAB = SLAB_R * SLAB_C   # 1300
CGROUPS, CPG = 4, 8      # channel groups of 8 -> staging partitions = 2*4=8
# Channels whose |x-w| + max runs entirely on VectorE (load balance vs ScalarE)
DVE_ORDER = (2, 8, 14, 20, 26)
DVE_CHANNELS = frozenset(DVE_ORDER)

F32 = mybir.dt.float32
BF16 = mybir.dt.bfloat16


def build_nc():
    nc = bacc.Bacc(trn_type="TRN2")

    x_slab = nc.declare_dram_parameter(
        "x_slab", [2, C, SLAB_R, SLAB_C], F32, isOutput=False
    )
    wbias = nc.declare_dram_parameter("wbias", [128, K], F32, isOutput=False)
    ones2 = nc.declare_dram_parameter("ones2", [2, 128], F32, isOutput=False)
    out = nc.declare_dram_parameter("out", [128, NHALF, WO], F32, isOutput=True)

    with TileContext(nc) as tc:
        with (
            tc.tile_pool(name="const", bufs=1) as cpool,
            tc.tile_pool(name="work", bufs=3) as wpool,
            tc.tile_pool(name="psum", bufs=1, space="PSUM") as ppool,
        ):
            wb = cpool.tile([128, K], F32)
            ones = cpool.tile([2, 128], F32)
            # Three stacked max-accumulators (one TT covers a 3-tap batch);
            # acc_lo tracks min(x-w) for the VectorE-only channels.
            acc3 = cpool.tile([128, 3, NHALF, WO], BF16)
            acc_lo = cpool.tile([128, NHALF, WO], BF16)
            accf = cpool.tile([128, NHALF, WO], BF16)
            acc32 = cpool.tile([128, NHALF, WO], F32)
            # Persistent SBUF copies of the VectorE-only channels' broadcasts
            # (decouples their STT stream from the PSUM slab pipeline).
            xd = cpool.tile([128, len(DVE_ORDER), SLAB_R, SLAB_C], F32)
            stage = cpool.tile([2, C // 2, SLAB_R, SLAB_C], F32)
            # Two persistent PSUM slabs, manually alternated per channel —
            # rotating pool slots would put >1 sem wait on the matmuls
            # (walrus allows only one on LDWEIGHTS).
            slab_a = ppool.tile([128, SLAB_R, SLAB_C], F32, tag="slab_a")
            slab_b = ppool.tile([128, SLAB_R, SLAB_C], F32, tag="slab_b")
            slabs = [slab_a, slab_b]

            # Single-queue SWDGE loads so consumers wait on one DMA sem.
            nc.gpsimd.dma_start(stage[:], x_slab[:, 0 : C // 2])
            nc.gpsimd.dma_start(wb[:], wbias[:])
            nc.gpsimd.dma_start(ones[:], ones2[:])
            nc.vector.memset(acc3[:], 0.0)
            nc.vector.memset(acc_lo[:], 0.0)

            # Dummy 1-column matmul: absorbs the `ones` DMA wait on PE so
            # real matmuls carry at most one sem wait (LDWEIGHTS limit).
            slab0_f = slabs[0].rearrange("p r c -> p (r c)")
            nc.tensor.matmul(
                slab0_f[:, 0:1], ones[:], ones[:, 0:1], start=True, stop=True
            )

            pending = []
            for c in range(C):
                if c == C // 2:
                    # Second half of the input channels (WAR on the PE reads
                    # is tracked by Tile; overlaps with compute of c=15).
                    nc.gpsimd.dma_start(stage[:], x_slab[:, C // 2 :])
                slab = slabs[c % 2]
                slab_f = slab.rearrange("p r c -> p (r c)")
                rhs = stage[:, c % (C // 2)].rearrange("p r c -> p (r c)")
                # Broadcast both halves of channel c to the 128 partitions.
                for off in range(0, SLAB, 512):
                    sz = min(512, SLAB - off)
                    nc.tensor.matmul(
                        slab_f[:, off : off + sz],
                        ones[:],
                        rhs[:, off : off + sz],
                        start=True,
                        stop=True,
                    )
                if c in DVE_CHANNELS:
                    # Evacuate the broadcast to SBUF; the STT pairs below are
                    # queued and interleaved between triplet TTs so VectorE
                    # work never holds a PSUM slab hostage.
                    idx = DVE_ORDER.index(c)
                    nc.vector.tensor_copy(xd[:, idx], slab[:])

                    def make_pair(idx, k, kh, kw):
                        def emit():
                            view = xd[:, idx, kh : kh + NHALF, kw : kw + WO]
                            # acc_hi = max(x-w, acc_hi); acc_lo = min(x-w, .)
                            nc.vector.scalar_tensor_tensor(
                                acc3[:, 0],
                                view,
                                wb[:, k : k + 1],
                                acc3[:, 0],
                                op0=mybir.AluOpType.subtract,
                                op1=mybir.AluOpType.max,
                            )
                            nc.vector.scalar_tensor_tensor(
                                acc_lo[:],
                                view,
                                wb[:, k : k + 1],
                                acc_lo[:],
                                op0=mybir.AluOpType.subtract,
                                op1=mybir.AluOpType.min,
                            )

                        return emit

                    for tap in range(KS * KS):
                        kh, kw = tap // KS, tap % KS
                        pending.append(
                            make_pair(idx, c * (KS * KS) + tap, kh, kw)
                        )
                else:
                    for t0 in range(0, KS * KS, 3):
                        tmp3 = wpool.tile([128, 3, NHALF, WO], BF16, tag="tmp")
                        for j in range(3):
                            tap = t0 + j
                            kh, kw = tap // KS, tap % KS
                            k = c * (KS * KS) + tap
                            # tmp3[:,j] = |w[o,k] - x_tap|
                            nc.scalar.activation(
                                tmp3[:, j],
                                slab[:, kh : kh + NHALF, kw : kw + WO],
                                mybir.ActivationFunctionType.Abs,
                                bias=wb[:, k : k + 1],
                                scale=-1.0,
                            )
                        nc.vector.tensor_tensor(
                            acc3[:], acc3[:], tmp3[:], op=mybir.AluOpType.max
                        )
                        if pending:
                            pending.pop(0)()

            for emit in pending:
                emit()
            # y = max(acc3[0..2], -acc_lo)
            nc.vector.tensor_tensor(
                accf[:], acc3[:, 0], acc3[:, 1], op=mybir.AluOpType.max
            )
            nc.vector.tensor_tensor(
                accf[:], accf[:], acc3[:, 2], op=mybir.AluOpType.max
            )
            nc.vector.scalar_tensor_tensor(
                accf[:],
                acc_lo[:],
                -1.0,
                accf[:],
                op0=mybir.AluOpType.mult,
                op1=mybir.AluOpType.max,
            )
            nc.scalar.copy(acc32[:], accf[:])
            nc.sync.dma_start(out[:], acc32[:])

    nc.compile()
    return nc


_NC_CACHE = {}


def _get_nc():
    if "nc" not in _NC_CACHE:
        _NC_CACHE["nc"] = build_nc()
    return _NC_CACHE["nc"]


def make_in_maps(inputs: np.ndarray, weights: np.ndarray):
    x = np.asarray(inputs, dtype=np.float32)
    w = np.asarray(weights, dtype=np.float32)
    assert x.shape == (B, C, H, W) and w.shape == (O, C, KS, KS)

    xp = np.zeros((B, C, H + 2 * PAD, W + 2 * PAD), np.float32)
    xp[:, :, PAD : PAD + H, PAD : PAD + W] = x
    half_a = xp[:, :, 0:SLAB_R, :]                    # (B, C, 26, 50)
    half_b = xp[:, :, NHALF : NHALF + SLAB_R, :]      # (B, C, 26, 50)
    halves = np.stack([half_a, half_b], axis=2)       # (B, C, 2, 26, 50)
    stage = halves.transpose(0, 2, 1, 3, 4)           # (B, 2, C, 26, 50)

    wf = w.reshape(O, K)
    wb = np.ascontiguousarray(np.tile(wf, (2, 1)))    # (128, K)
    ones2 = np.zeros((2, 128), np.float32)
    ones2[0, :64] = 1.0
    ones2[1, 64:] = 1.0

    return [
        {
            "x_slab": np.ascontiguousarray(stage[b]),
            "wbias": wb,
            "ones2": ones2,
        }
        for b in range(B)
    ]


def assemble_output(results):
    y = np.empty((B, O, HO, WO), np.float32)
    for b in range(B):
        o = results[b]["out"]
        y[b, :, :NHALF, :] = o[0:64]
        y[b, :, NHALF:, :] = o[64:128]
    return y


def launch(inputs: np.ndarray, weights: np.ndarray, trace: bool = False):
    """Run on 8 NeuronCores; returns (y, BassKernelResults)."""
    in_maps = make_in_maps(inputs, weights)
    res = run_bass_kernel_spmd(
        _get_nc(), in_maps, list(range(B)), trace=trace
    )
    return assemble_output(res.results), res


def kernel(inputs: np.ndarray, weights: np.ndarray) -> np.ndarray:
    y, _ = launch(inputs, weights, trace=False)
    return y
